# revision 1
# baseline (speedup 1.0000x reference)
"""Trainium2 Bass kernel for FlaxSapama (Llama-style) attention block.

Strategy: tensor-parallel over heads across 8 NeuronCores.
Core m owns Q heads [4m..4m+4) and KV head m (GQA group of 4), plus the
matching slice of Wo rows. Each core computes a full [T, HIDDEN] partial
output (its heads' contribution through Wo); the host sums the 8 partials.

Per-core pipeline (all matmuls bf16 inputs, fp32 PSUM accumulation):
  1. QKV projections computed transposed: qT/kT [head_dim, tokens] via
     lhsT=W tile, rhs=hidden^T tile; RoPE applied on PSUM evacuation.
     V is DMA-transposed to [tokens, head_dim] tiles for the PV matmul.
  2. Attention with scores computed transposed: S^T[k,q] tiles; softmax
     denominators via ones-matmul accumulation in PSUM; causal masking via
     additive mask tiles on diagonal blocks only; exp on ScalarE with
     per-partition key-padding bias; 1/denominator broadcast across
     partitions via gpsimd.partition_broadcast.
  3. Output projection accumulating 4 head slices per PSUM tile.

Tiles are split per (head, batch) so the Tile scheduler can overlap the
three phases across batches.
"""

import math

import numpy as np
import ml_dtypes

import concourse.bacc as bacc
import concourse.tile as tile
import concourse.mybir as mybir
from concourse.bass_utils import run_bass_kernel_spmd

BF16 = mybir.dt.bfloat16
F32 = mybir.dt.float32
NP_BF16 = ml_dtypes.bfloat16

HIDDEN = 4096
N_HEADS = 32
N_KV = 8
HD = 128          # head dim
MAX_POS = 4096
B, S = 2, 2048
T = B * S         # 4096 tokens
NCORES = 8
HPC = N_HEADS // NCORES      # 4 q heads per core
KT = HIDDEN // 128           # 32 contraction tiles for projections
NB = T // 512                # 8 token blocks of 512
TT = T // 128                # 32 token tiles of 128
SKT = S // 128               # 16 k-pos tiles per batch
NEG = -1.0e9

_PROGRAM = None


def _build_program():
    nc = bacc.Bacc(None, target_bir_lowering=False)

    # all inputs partition-major so DMA runs are >=1KB contiguous per partition
    hst_d = nc.dram_tensor("hst", [128, KT, T], BF16, kind="ExternalInput")
    sin_d = nc.dram_tensor("sint", [128, T], BF16, kind="ExternalInput")
    cos_d = nc.dram_tensor("cost", [128, T], BF16, kind="ExternalInput")
    wq_d = nc.dram_tensor("wq", [128, HPC, KT, HD], BF16, kind="ExternalInput")
    wk_d = nc.dram_tensor("wk", [128, KT, HD], BF16, kind="ExternalInput")
    wv_d = nc.dram_tensor("wv", [128, KT, HD], BF16, kind="ExternalInput")
    wo_d = nc.dram_tensor("wo", [128, HPC, HIDDEN], BF16, kind="ExternalInput")
    masks_d = nc.dram_tensor("masks", [128, 4, 512], BF16, kind="ExternalInput")
    kb_d = nc.dram_tensor("kbias", [128, TT], F32, kind="ExternalInput")
    out_d = nc.dram_tensor("out", [TT, 128, HIDDEN], F32, kind="ExternalOutput")

    with tile.TileContext(nc) as tc:
        # one PSUM pool for the whole kernel: a shared "big" tag (6 banks) lets
        # consecutive phases overlap through slot recycling instead of
        # serializing on pool region reuse; "den" gets the other 2 banks
        with tc.tile_pool(name="qkv", bufs=1) as pool_qkv, \
             tc.tile_pool(name="psA", bufs=4, space="PSUM") as psA:
            # per-(head, batch) tiles so phases can overlap across batches
            qT = [[pool_qkv.tile([128, S], BF16, name=f"qT_{h}_{b}")
                   for b in range(B)] for h in range(HPC)]
            kTt = [pool_qkv.tile([128, S], BF16, name=f"kT_{b}") for b in range(B)]
            vt = [pool_qkv.tile([128, SKT, HD], BF16, name=f"v_{b}") for b in range(B)]
            aout = [[pool_qkv.tile([128, S], BF16, name=f"ao_{h}_{b}")
                     for b in range(B)] for h in range(HPC)]
            masks_sb = pool_qkv.tile([128, 4, 512], BF16)
            kb_sb = pool_qkv.tile([128, TT], F32)
            ones_sb = pool_qkv.tile([128, 1], BF16)
            nc.vector.memset(ones_sb, 1.0)

            # ---------------- Phase 1: QKV projections + RoPE ----------------
            with tc.tile_pool(name="p1w", bufs=1) as p1w, \
                 tc.tile_pool(name="p1h", bufs=2) as p1h, \
                 tc.tile_pool(name="p1t", bufs=2) as p1t:

                def load_hst(nb):
                    tok = slice(nb * 512, (nb + 1) * 512)
                    tiles = [p1h.tile([128, KT // 4, 512], BF16, tag=f"hst{q}",
                                      name=f"hst{q}_{nb}")
                             for q in range(4)]
                    for q in range(4):
                        nc.sync.dma_start(
                            out=tiles[q], in_=hst_d[:, q * 8:(q + 1) * 8, tok])
                    sin_t = p1h.tile([128, 512], BF16, tag="sin", name=f"sin_{nb}")
                    cos_t = p1h.tile([128, 512], BF16, tag="cos", name=f"cos_{nb}")
                    nc.sync.dma_start(out=sin_t, in_=sin_d[:, tok])
                    nc.sync.dma_start(out=cos_t, in_=cos_d[:, tok])
                    return tiles, sin_t, cos_t

                wq_sb = [p1w.tile([128, KT, HD], BF16, name=f"wq_{h}")
                         for h in range(HPC)]
                wk_sb = p1w.tile([128, KT, HD], BF16)
                wv_a = p1w.tile([128, 8, HD], BF16)
                wv_b = p1w.tile([128, KT - 8, HD], BF16)
                # DMA emission order matches first-chain consumption exactly
                nc.sync.dma_start(out=wv_a, in_=wv_d[:, 0:8, :])
                h0 = [p1h.tile([128, KT // 4, 512], BF16, tag=f"hst{q}",
                               name=f"hst{q}_0") for q in range(4)]
                nc.sync.dma_start(out=h0[0], in_=hst_d[:, 0:8, 0:512])
                nc.sync.dma_start(out=wv_b, in_=wv_d[:, 8:KT, :])
                nc.sync.dma_start(out=h0[1], in_=hst_d[:, 8:16, 0:512])
                nc.sync.dma_start(out=wk_sb, in_=wk_d[:, :, :])
                nc.sync.dma_start(out=h0[2], in_=hst_d[:, 16:24, 0:512])
                nc.sync.dma_start(out=h0[3], in_=hst_d[:, 24:32, 0:512])
                sin_0 = p1h.tile([128, 512], BF16, tag="sin", name="sin_0")
                cos_0 = p1h.tile([128, 512], BF16, tag="cos", name="cos_0")
                nc.sync.dma_start(out=sin_0, in_=sin_d[:, 0:512])
                nc.sync.dma_start(out=cos_0, in_=cos_d[:, 0:512])
                nb0_tiles = (h0, sin_0, cos_0)
                for h in range(HPC):
                    nc.sync.dma_start(out=wq_sb[h], in_=wq_d[:, h, :, :])
                nc.sync.dma_start(out=masks_sb, in_=masks_d[:, :, :])
                nc.sync.dma_start(out=kb_sb, in_=kb_d[:, :])

                for nb in range(NB):
                    b, qb = nb // 4, nb % 4
                    bsl = slice(qb * 512, (qb + 1) * 512)
                    hst_t, sin_t, cos_t = nb0_tiles if nb == 0 else load_hst(nb)

                    for m in ((5, 4, 0, 1, 2, 3) if nb < NB - 1 else (0, 1, 2, 3, 4, 5)):
                        psum = psA.tile([128, 512], F32, tag="big", name=f"pj_{nb}_{m}")
                        for kt in range(KT):
                            if m < HPC:
                                lhsT = wq_sb[m][:, kt, :]
                            elif m == HPC:
                                lhsT = wk_sb[:, kt, :]
                            elif kt < 8:
                                lhsT = wv_a[:, kt, :]
                            else:
                                lhsT = wv_b[:, kt - 8, :]
                            nc.tensor.matmul(psum[:, :], lhsT=lhsT,
                                             rhs=hst_t[kt // 8][:, kt % 8, :],
                                             start=(kt == 0), stop=(kt == KT - 1))
                        if m <= HPC:
                            # rope: out = x*cos + shift_half(x)*sin' (sign in sin')
                            # cross-half reads straight from PSUM (SB-SB ops need
                            # equal base partitions; PSUM+SB is exempt)
                            tmp = p1t.tile([128, 512], BF16, tag="tmp")
                            nc.vector.tensor_mul(tmp[0:64, :], psum[64:128, :], sin_t[0:64, :])
                            nc.vector.tensor_mul(tmp[64:128, :], psum[0:64, :], sin_t[64:128, :])
                            t2 = p1t.tile([128, 512], BF16, tag="t2")
                            nc.vector.tensor_mul(t2, psum[:, :], cos_t)
                            dest = qT[m][b][:, bsl] if m < HPC else kTt[b][:, bsl]
                            nc.vector.tensor_add(dest, t2, tmp)
                        else:
                            v_bf = p1t.tile([128, 512], BF16, tag="vbf")
                            nc.scalar.copy(out=v_bf, in_=psum[:, :])
                            for j in range(4):
                                nc.sync.dma_start_transpose(
                                    out=vt[b][:, qb * 4 + j, :],
                                    in_=v_bf[:, j * 128:(j + 1) * 128])

            # ---------------- Phases 2+3 ----------------
            # p3 pools open (and wo loads) before p2 pools so the wo DMA only
            # waits on phase-1 readers, not on all of phase 2
            with tc.tile_pool(name="p3c", bufs=1) as p3c, \
                 tc.tile_pool(name="p3t", bufs=4) as p3t, \
                 tc.tile_pool(name="p2c", bufs=1) as p2c, \
                 tc.tile_pool(name="p2t", bufs=4) as p2t:
                wo_sb = p3c.tile([128, HPC, HIDDEN], BF16)
                for h in range(HPC):
                    nc.sync.dma_start(out=wo_sb[:, h, :], in_=wo_d[:, h, :])
                def emit_s(b, h, qb, kt):
                    # diagonal tiles (kt-4qb = o > 0): columns < 128*o are
                    # fully causal-masked -> skip them entirely
                    o_off = kt - 4 * qb
                    c0 = 128 * o_off if o_off > 0 else 0
                    cs = slice(c0, 512)
                    s_ps = psA.tile([128, 512], F32, tag="big",
                                    name=f"s_{b}_{h}_{qb}_{kt}")
                    nc.tensor.matmul(
                        s_ps[:, cs],
                        lhsT=kTt[b][:, kt * 128:(kt + 1) * 128],
                        rhs=qT[h][b][:, qb * 512 + c0:(qb + 1) * 512],
                        start=True, stop=True)
                    if o_off >= 0:
                        # only the 128-col boundary chunk holds the causal
                        # staircase; columns >= c0+128 are fully valid (mask 0)
                        cm = slice(c0, c0 + 128)
                        nc.vector.tensor_add(s_ps[:, cm], s_ps[:, cm],
                                             masks_sb[:, o_off, cm])
                    p_bf = p2t.tile([128, 512], BF16, tag="p", bufs=10)
                    gk = b * SKT + kt
                    nc.scalar.activation(
                        out=p_bf[:, cs], in_=s_ps[:, cs],
                        func=mybir.ActivationFunctionType.Exp,
                        bias=kb_sb[:, gk:gk + 1], scale=1.0)
                    return b, h, qb, kt, p_bf, cs

                acc = {}
                tails = []

                def emit_dp(b, h, qb, kt, p_bf, cs):
                    nkt = 4 * (qb + 1)
                    if kt == 0:
                        acc[(b, h, qb)] = (
                            psA.tile([128, 512], F32, tag="obank", bufs=2,
                                     name=f"o_{b}_{h}_{qb}"),
                            psA.tile([1, 512], F32, tag="den", bufs=2,
                                     name=f"den_{b}_{h}_{qb}"))
                    o_ps, den_ps = acc[(b, h, qb)]
                    nc.tensor.matmul(den_ps[:, cs], lhsT=ones_sb[:, :],
                                     rhs=p_bf[:, cs],
                                     start=(kt == 0), stop=(kt == nkt - 1))
                    nc.tensor.matmul(o_ps[:, cs], lhsT=vt[b][:, kt, :],
                                     rhs=p_bf[:, cs],
                                     start=(kt == 0), stop=(kt == nkt - 1))
                    if kt == nkt - 1:
                        tails.append((b, h, qb))
                    elif kt == 2 and tails:
                        # deferred: run the previous qb's normalization on DVE
                        # after this qb's first mask-adds, not before them
                        emit_tail(*tails.pop(0))

                def emit_tail(b, h, qb):
                    o_ps, den_ps = acc.pop((b, h, qb))
                    qsl = slice(qb * 512, (qb + 1) * 512)
                    recip = p2t.tile([1, 512], F32, tag="recip")
                    nc.vector.reciprocal(recip, den_ps[:, :])
                    rb = p2t.tile([128, 512], F32, tag="rb")
                    nc.gpsimd.partition_broadcast(rb[:, :], recip[:, :])
                    nc.vector.tensor_mul(aout[h][b][:, qsl], o_ps[:, :], rb[:, :])

                # one software pipeline across the whole attention phase:
                # den/pv (and each qb's normalization tail) lag the scores
                # matmul by 5 iterations so PE never waits on the exp (ACT)
                pend = []
                for b in range(B):
                    for h in range(HPC):
                        for qb in range(4):
                            for kt in range(4 * (qb + 1)):
                                pend.append(emit_s(b, h, qb, kt))
                                if len(pend) > 7:
                                    emit_dp(*pend.pop(0))
                for args in pend:
                    emit_dp(*args)
                while tails:
                    emit_tail(*tails.pop(0))

            # ---------------- Phase 3: output projection ----------------
                oo_tags = (("big", None), ("big", None), ("obank", 2), ("den", 2))
                for tb in range(TT):
                    b, tloc = tb // SKT, tb % SKT
                    for ob in range(8):
                        tg, bf = oo_tags[(tb * 8 + ob) % 4]
                        o_ps3 = psA.tile([128, 512], F32, tag=tg, bufs=bf,
                                         name=f"oo_{tb}_{ob}")
                        for h in range(HPC):
                            nc.tensor.matmul(
                                o_ps3[:, :],
                                lhsT=aout[h][b][:, tloc * 128:(tloc + 1) * 128],
                                rhs=wo_sb[:, h, ob * 512:(ob + 1) * 512],
                                start=(h == 0), stop=(h == HPC - 1))
                        osb = p3t.tile([128, 512], F32, tag="osb", bufs=8)
                        if (tb * 8 + ob) % 2 == 0:
                            nc.scalar.copy(out=osb, in_=o_ps3[:, :])
                        else:
                            nc.vector.tensor_copy(out=osb, in_=o_ps3[:, :])
                        nc.sync.dma_start(out=out_d[tb, :, ob * 512:(ob + 1) * 512],
                                          in_=osb)
    nc.compile()
    return nc


def _rope_tables():
    freqs = np.einsum("i,j->ij", np.arange(MAX_POS),
                      1.0 / 10000 ** (np.arange(0, HD, 2) / HD)).astype("float32")
    emb = np.concatenate((freqs, freqs), axis=-1)  # [pos, HD]
    return np.sin(emb), np.cos(emb)


def _prep_inputs(hidden_states, attention_mask, position_ids, Wq, Wk, Wv, Wo):
    hs = np.ascontiguousarray(np.asarray(hidden_states, dtype=np.float32))
    am = np.asarray(attention_mask, dtype=np.float32)
    pid = np.asarray(position_ids).astype(np.int64).reshape(-1)

    sin, cos = _rope_tables()
    sinT = np.ascontiguousarray(sin[pid].T)   # [HD, T]
    cosT = np.ascontiguousarray(cos[pid].T)
    sinT[0:HD // 2] *= -1.0                    # fold rotate-half sign
    sin_in = sinT.astype(NP_BF16)
    cos_in = cosT.astype(NP_BF16)

    hsT = hs.reshape(T, HIDDEN).T                          # [HIDDEN, T]
    hst_in = np.ascontiguousarray(
        hsT.reshape(KT, 128, T).transpose(1, 0, 2)).astype(NP_BF16)  # [128, KT, T]

    # causal masks for diagonal blocks: allowed iff c >= 128*o + r
    r = np.arange(128)[:, None]
    c = np.arange(512)[None, :]
    masks = np.stack([np.where(c >= 128 * o + r, 0.0, NEG) for o in range(4)])
    masks = np.ascontiguousarray(masks.transpose(1, 0, 2)).astype(NP_BF16)  # [128,4,512]

    kb = np.where(am.reshape(-1) > 0, 0.0, NEG).astype(np.float32)
    kb_in = np.ascontiguousarray(kb.reshape(TT, 128).T)   # [128, TT]

    scale = 1.0 / math.sqrt(HD)
    Wq = np.asarray(Wq, dtype=np.float32) * scale
    Wk = np.asarray(Wk, dtype=np.float32)
    Wv = np.asarray(Wv, dtype=np.float32)
    Wo = np.asarray(Wo, dtype=np.float32)

    in_maps = []
    for m in range(NCORES):
        wq_m = np.ascontiguousarray(Wq[:, m * HPC * HD:(m + 1) * HPC * HD])
        wk_m = np.ascontiguousarray(Wk[:, m * HD:(m + 1) * HD])
        wv_m = np.ascontiguousarray(Wv[:, m * HD:(m + 1) * HD])
        wo_m = np.ascontiguousarray(Wo[m * HPC * HD:(m + 1) * HPC * HD, :])
        in_maps.append({
            "hst": hst_in,
            "sint": sin_in,
            "cost": cos_in,
            # [128, HPC, KT, HD]: partition-major, per-head blocked
            "wq": np.ascontiguousarray(
                wq_m.reshape(KT, 128, HPC, HD).transpose(1, 2, 0, 3)).astype(NP_BF16),
            "wk": np.ascontiguousarray(
                wk_m.reshape(KT, 128, HD).transpose(1, 0, 2)).astype(NP_BF16),
            "wv": np.ascontiguousarray(
                wv_m.reshape(KT, 128, HD).transpose(1, 0, 2)).astype(NP_BF16),
            "wo": np.ascontiguousarray(
                wo_m.reshape(HPC, 128, HIDDEN).transpose(1, 0, 2)).astype(NP_BF16),
            "masks": masks,
            "kbias": kb_in,
        })
    return in_maps


def get_program():
    global _PROGRAM
    if _PROGRAM is None:
        _PROGRAM = _build_program()
    return _PROGRAM


def kernel(**inputs):
    nc = get_program()
    in_maps = _prep_inputs(
        inputs["hidden_states"], inputs["attention_mask"], inputs["position_ids"],
        inputs["Wq"], inputs["Wk"], inputs["Wv"], inputs["Wo"])
    res = run_bass_kernel_spmd(nc, in_maps, core_ids=list(range(NCORES)))
    acc = np.zeros((TT, 128, HIDDEN), dtype=np.float32)
    for r in res.results:
        acc += r["out"]
    return acc.reshape(B, S, HIDDEN)



# revision 2
# speedup vs baseline: 1.1418x; 1.1418x over previous
"""Trainium2 Bass kernel for FlaxSapama (Llama-style) attention block.

Strategy: tensor-parallel over heads across 8 NeuronCores.
Core m owns Q heads [4m..4m+4) and KV head m (GQA group of 4), plus the
matching slice of Wo rows. Each core computes a full [T, HIDDEN] partial
output (its heads' contribution through Wo); the host sums the 8 partials.

Per-core pipeline:
  1. QKV projections in fp8e4 DoubleRow mode with hi/lo error
     compensation: X ~ X_hi + X_lo (both fp8), W.T@X ~ Whi.T@Xhi +
     Wlo.T@Xhi + Whi.T@Xlo (lo.lo dropped). The three plane-products per
     contraction tile pack into 1.5 DoubleRow instructions (2 planes each
     at 0.5 cycles/row), i.e. 0.75x the bf16 matmul cycles at ~bf16
     accuracy. Weights pre-split on host (scaled x1024 into fp8 range,
     compensated via sin/cos tables, V-copy scale, and host divide);
     hidden states pre-split on host. RoPE applied on PSUM evacuation.
  2. Attention with scores computed transposed in bf16: S^T[k,q] tiles;
     softmax denominators via ones-matmul accumulation in PSUM; causal
     masking via additive mask tiles on diagonal blocks only; exp on
     ScalarE with per-partition key-padding bias; 1/denominator broadcast
     across partitions via gpsimd.partition_broadcast. Normalized head
     outputs are written as fp8 hi/lo planes (x16, folded into the
     reciprocal via the ones value) for phase 3.
  3. Output projection in fp8e4 DoubleRow with hi/lo planes for both
     aout and Wo: per head-pair, one hi.hi instruction plus one cross
     instruction per head (0.75x bf16 cycles).

Tiles are split per (head, batch) so the Tile scheduler can overlap the
three phases across batches.
"""

import math

import numpy as np
import ml_dtypes

import concourse.bacc as bacc
import concourse.tile as tile
import concourse.mybir as mybir
from concourse.bass_utils import run_bass_kernel_spmd

BF16 = mybir.dt.bfloat16
F32 = mybir.dt.float32
FP8 = mybir.dt.float8e4
NP_BF16 = ml_dtypes.bfloat16
NP_FP8 = ml_dtypes.float8_e4m3
DR = mybir.MatmulPerfMode.DoubleRow

HIDDEN = 4096
N_HEADS = 32
N_KV = 8
HD = 128          # head dim
MAX_POS = 4096
B, S = 2, 2048
T = B * S         # 4096 tokens
NCORES = 8
HPC = N_HEADS // NCORES      # 4 q heads per core
KT = HIDDEN // 128           # 32 contraction tiles for projections
NB = T // 512                # 8 token blocks of 512
TT = T // 128                # 32 token tiles of 128
SKT = S // 128               # 16 k-pos tiles per batch
NEG = -1.0e9

WS = 1024.0       # weight prescale into fp8 range (exact power of 2)
SA = 16.0         # aout prescale (folded into ones value)
ONES_VAL = 1.0 / SA
KB_SHIFT = -4.0 * math.log(2.0)   # global exp shift (cancels in softmax)

_PROGRAM = None


def _build_program():
    nc = bacc.Bacc(None, target_bir_lowering=False)

    # all inputs partition-major so DMA runs are >=512B contiguous per
    # partition; fp8 tensors carry (hi, lo) planes for error compensation
    hst_d = nc.dram_tensor("hst", [128, KT, 2, T], FP8, kind="ExternalInput")
    sin_d = nc.dram_tensor("sint", [128, T], BF16, kind="ExternalInput")
    cos_d = nc.dram_tensor("cost", [128, T], BF16, kind="ExternalInput")
    wq_d = nc.dram_tensor("wq", [128, HPC, KT, 2, HD], FP8, kind="ExternalInput")
    wk_d = nc.dram_tensor("wk", [128, KT, 2, HD], FP8, kind="ExternalInput")
    wv_d = nc.dram_tensor("wv", [128, KT, 2, HD], FP8, kind="ExternalInput")
    wo_d = nc.dram_tensor("wo", [128, HPC, 2, HIDDEN], FP8, kind="ExternalInput")
    masks_d = nc.dram_tensor("masks", [128, 4, 512], BF16, kind="ExternalInput")
    kb_d = nc.dram_tensor("kbias", [128, TT], F32, kind="ExternalInput")
    out_d = nc.dram_tensor("out", [TT, 128, HIDDEN], F32, kind="ExternalOutput")

    with tile.TileContext(nc) as tc:
        # one PSUM pool for the whole kernel: a shared "big" tag lets
        # consecutive phases overlap through slot recycling instead of
        # serializing on pool region reuse; "den" gets the other banks
        with tc.tile_pool(name="qkv", bufs=1) as pool_qkv, \
             tc.tile_pool(name="psA", bufs=4, space="PSUM") as psA:
            # per-(head, batch) tiles so phases can overlap across batches
            qT = [[pool_qkv.tile([128, S], BF16, name=f"qT_{h}_{b}")
                   for b in range(B)] for h in range(HPC)]
            kTt = [pool_qkv.tile([128, S], BF16, name=f"kT_{b}") for b in range(B)]
            vt = [pool_qkv.tile([128, SKT, HD], BF16, name=f"v_{b}") for b in range(B)]
            # normalized head outputs as fp8 planes: per head j0=lo, j1=hi
            aout8 = [pool_qkv.tile([128, HPC, 2, S], FP8, name=f"ao8_{b}")
                     for b in range(B)]
            masks_sb = pool_qkv.tile([128, 4, 512], BF16)
            kb_sb = pool_qkv.tile([128, TT], F32)
            ones_sb = pool_qkv.tile([128, 1], BF16)
            nc.vector.memset(ones_sb, ONES_VAL)

            # ---------------- Phase 1: QKV projections + RoPE ----------------
            with tc.tile_pool(name="p1w", bufs=1) as p1w, \
                 tc.tile_pool(name="p1h", bufs=2) as p1h, \
                 tc.tile_pool(name="p1t", bufs=2) as p1t:

                def load_hst(nb):
                    tok = slice(nb * 512, (nb + 1) * 512)
                    tiles = [p1h.tile([128, KT // 4, 2, 512], FP8, tag=f"hst{q}",
                                      name=f"hst{q}_{nb}")
                             for q in range(4)]
                    for q in range(4):
                        nc.sync.dma_start(
                            out=tiles[q], in_=hst_d[:, q * 8:(q + 1) * 8, :, tok])
                    sin_t = p1h.tile([128, 512], BF16, tag="sin", name=f"sin_{nb}")
                    cos_t = p1h.tile([128, 512], BF16, tag="cos", name=f"cos_{nb}")
                    nc.sync.dma_start(out=sin_t, in_=sin_d[:, tok])
                    nc.sync.dma_start(out=cos_t, in_=cos_d[:, tok])
                    return tiles, sin_t, cos_t

                wq_sb = [p1w.tile([128, KT, 2, HD], FP8, name=f"wq_{h}")
                         for h in range(HPC)]
                wk_sb = p1w.tile([128, KT, 2, HD], FP8)
                wv_a = p1w.tile([128, 8, 2, HD], FP8)
                wv_b = p1w.tile([128, KT - 8, 2, HD], FP8)
                # DMA emission order matches first-chain consumption exactly
                nc.sync.dma_start(out=wv_a, in_=wv_d[:, 0:8, :, :])
                h0 = [p1h.tile([128, KT // 4, 2, 512], FP8, tag=f"hst{q}",
                               name=f"hst{q}_0") for q in range(4)]
                nc.sync.dma_start(out=h0[0], in_=hst_d[:, 0:8, :, 0:512])
                nc.sync.dma_start(out=wv_b, in_=wv_d[:, 8:KT, :, :])
                nc.sync.dma_start(out=h0[1], in_=hst_d[:, 8:16, :, 0:512])
                nc.sync.dma_start(out=wk_sb, in_=wk_d[:, :, :, :])
                nc.sync.dma_start(out=h0[2], in_=hst_d[:, 16:24, :, 0:512])
                nc.sync.dma_start(out=h0[3], in_=hst_d[:, 24:32, :, 0:512])
                sin_0 = p1h.tile([128, 512], BF16, tag="sin", name="sin_0")
                cos_0 = p1h.tile([128, 512], BF16, tag="cos", name="cos_0")
                nc.sync.dma_start(out=sin_0, in_=sin_d[:, 0:512])
                nc.sync.dma_start(out=cos_0, in_=cos_d[:, 0:512])
                nb0_tiles = (h0, sin_0, cos_0)
                for h in range(HPC):
                    nc.sync.dma_start(out=wq_sb[h], in_=wq_d[:, h, :, :, :])
                nc.sync.dma_start(out=masks_sb, in_=masks_d[:, :, :])
                nc.sync.dma_start(out=kb_sb, in_=kb_d[:, :])

                for nb in range(NB):
                    b, qb = nb // 4, nb % 4
                    bsl = slice(qb * 512, (qb + 1) * 512)
                    hst_t, sin_t, cos_t = nb0_tiles if nb == 0 else load_hst(nb)

                    for m in ((5, 4, 0, 1, 2, 3) if nb < NB - 1 else (0, 1, 2, 3, 4, 5)):
                        if m < HPC:
                            w_of = lambda sl3, sl2: wq_sb[m][:, sl3, sl2, :]
                        elif m == HPC:
                            w_of = lambda sl3, sl2: wk_sb[:, sl3, sl2, :]
                        else:
                            def w_of(sl3, sl2):
                                st = sl3 if isinstance(sl3, int) else sl3.start
                                if st < 8:
                                    return wv_a[:, sl3, sl2, :]
                                sl3b = (sl3 - 8 if isinstance(sl3, int)
                                        else slice(sl3.start - 8, sl3.stop - 8))
                                return wv_b[:, sl3b, sl2, :]
                        psum = psA.tile([128, 512], F32, tag="big", name=f"pj_{nb}_{m}")
                        # fp8 DoubleRow: per kt pair, one hi.hi instruction
                        # (strided planes) + one cross instruction per kt
                        # (contiguous (lo,hi)x(hi,lo) planes)
                        for ktp in range(0, KT, 2):
                            q, r = ktp // 8, ktp % 8
                            ht = hst_t[q]
                            nc.tensor.matmul(psum[:, :],
                                             lhsT=w_of(slice(ktp, ktp + 2), 1),
                                             rhs=ht[:, r:r + 2, 0, :],
                                             perf_mode=DR,
                                             start=(ktp == 0), stop=False)
                            nc.tensor.matmul(psum[:, :],
                                             lhsT=w_of(ktp, slice(0, 2)),
                                             rhs=ht[:, r, 0:2, :],
                                             perf_mode=DR, start=False, stop=False)
                            nc.tensor.matmul(psum[:, :],
                                             lhsT=w_of(ktp + 1, slice(0, 2)),
                                             rhs=ht[:, r + 1, 0:2, :],
                                             perf_mode=DR, start=False,
                                             stop=(ktp == KT - 2))
                        if m <= HPC:
                            # rope: out = x*cos + shift_half(x)*sin' (sign in
                            # sin'; 1/WS folded into the sin/cos tables).
                            # cross-half reads straight from PSUM (SB-SB ops
                            # need equal base partitions; PSUM+SB is exempt)
                            tmp = p1t.tile([128, 512], BF16, tag="tmp")
                            nc.vector.tensor_mul(tmp[0:64, :], psum[64:128, :], sin_t[0:64, :])
                            nc.vector.tensor_mul(tmp[64:128, :], psum[0:64, :], sin_t[64:128, :])
                            t2 = p1t.tile([128, 512], BF16, tag="t2")
                            nc.vector.tensor_mul(t2, psum[:, :], cos_t)
                            dest = qT[m][b][:, bsl] if m < HPC else kTt[b][:, bsl]
                            nc.vector.tensor_add(dest, t2, tmp)
                        else:
                            v_bf = p1t.tile([128, 512], BF16, tag="vbf")
                            nc.scalar.mul(v_bf, psum[:, :], 1.0 / WS)
                            for j in range(4):
                                nc.sync.dma_start_transpose(
                                    out=vt[b][:, qb * 4 + j, :],
                                    in_=v_bf[:, j * 128:(j + 1) * 128])

            # ---------------- Phases 2+3 ----------------
            # p3 pools open (and wo loads) before p2 pools so the wo DMA only
            # waits on phase-1 readers, not on all of phase 2
            with tc.tile_pool(name="p3c", bufs=1) as p3c, \
                 tc.tile_pool(name="p3t", bufs=4) as p3t, \
                 tc.tile_pool(name="p2c", bufs=1) as p2c, \
                 tc.tile_pool(name="p2t", bufs=4) as p2t:
                wo_sb = p3c.tile([128, HPC, 2, HIDDEN], FP8)
                for h in range(HPC):
                    nc.sync.dma_start(out=wo_sb[:, h, :, :], in_=wo_d[:, h, :, :])
                def emit_s(b, h, qb, kt):
                    # diagonal tiles (kt-4qb = o > 0): columns < 128*o are
                    # fully causal-masked -> skip them entirely
                    o_off = kt - 4 * qb
                    c0 = 128 * o_off if o_off > 0 else 0
                    cs = slice(c0, 512)
                    s_ps = psA.tile([128, 512], F32, tag="big",
                                    name=f"s_{b}_{h}_{qb}_{kt}")
                    nc.tensor.matmul(
                        s_ps[:, cs],
                        lhsT=kTt[b][:, kt * 128:(kt + 1) * 128],
                        rhs=qT[h][b][:, qb * 512 + c0:(qb + 1) * 512],
                        start=True, stop=True)
                    if o_off >= 0:
                        # only the 128-col boundary chunk holds the causal
                        # staircase; columns >= c0+128 are fully valid (mask 0)
                        cm = slice(c0, c0 + 128)
                        nc.vector.tensor_add(s_ps[:, cm], s_ps[:, cm],
                                             masks_sb[:, o_off, cm])
                    p_bf = p2t.tile([128, 512], BF16, tag="p", bufs=10)
                    gk = b * SKT + kt
                    nc.scalar.activation(
                        out=p_bf[:, cs], in_=s_ps[:, cs],
                        func=mybir.ActivationFunctionType.Exp,
                        bias=kb_sb[:, gk:gk + 1], scale=1.0)
                    return b, h, qb, kt, p_bf, cs

                acc = {}
                tails = []

                def emit_dp(b, h, qb, kt, p_bf, cs):
                    nkt = 4 * (qb + 1)
                    if kt == 0:
                        acc[(b, h, qb)] = (
                            psA.tile([128, 512], F32, tag="obank", bufs=2,
                                     name=f"o_{b}_{h}_{qb}"),
                            psA.tile([1, 512], F32, tag="den", bufs=2,
                                     name=f"den_{b}_{h}_{qb}"))
                    o_ps, den_ps = acc[(b, h, qb)]
                    nc.tensor.matmul(den_ps[:, cs], lhsT=ones_sb[:, :],
                                     rhs=p_bf[:, cs],
                                     start=(kt == 0), stop=(kt == nkt - 1))
                    nc.tensor.matmul(o_ps[:, cs], lhsT=vt[b][:, kt, :],
                                     rhs=p_bf[:, cs],
                                     start=(kt == 0), stop=(kt == nkt - 1))
                    if kt == nkt - 1:
                        tails.append((b, h, qb))
                    elif kt == 2 and tails:
                        # deferred: run the previous qb's normalization on DVE
                        # after this qb's first mask-adds, not before them
                        emit_tail(*tails.pop(0))

                def emit_tail(b, h, qb):
                    o_ps, den_ps = acc.pop((b, h, qb))
                    qsl = slice(qb * 512, (qb + 1) * 512)
                    recip = p2t.tile([1, 512], F32, tag="recip")
                    nc.vector.reciprocal(recip, den_ps[:, :])
                    rb = p2t.tile([128, 512], F32, tag="rb")
                    nc.gpsimd.partition_broadcast(rb[:, :], recip[:, :])
                    nrm = p2t.tile([128, 512], BF16, tag="nrm")
                    nc.vector.tensor_mul(nrm, o_ps[:, :], rb[:, :])
                    # fp8 hi/lo planes of the (x SA) normalized output
                    hi = aout8[b][:, h, 1, qsl]
                    nc.scalar.copy(out=hi, in_=nrm)
                    nc.vector.tensor_sub(aout8[b][:, h, 0, qsl], nrm, hi)

                # one software pipeline across the whole attention phase:
                # den/pv (and each qb's normalization tail) lag the scores
                # matmul by 5 iterations so PE never waits on the exp (ACT)
                pend = []
                for b in range(B):
                    for h in range(HPC):
                        for qb in range(4):
                            for kt in range(4 * (qb + 1)):
                                pend.append(emit_s(b, h, qb, kt))
                                if len(pend) > 7:
                                    emit_dp(*pend.pop(0))
                for args in pend:
                    emit_dp(*args)
                while tails:
                    emit_tail(*tails.pop(0))

            # ---------------- Phase 3: output projection ----------------
                oo_tags = (("big", None), ("big", None), ("obank", 2), ("den", 2))
                for tb in range(TT):
                    b, tloc = tb // SKT, tb % SKT
                    tsl = slice(tloc * 128, (tloc + 1) * 128)
                    for ob in range(8):
                        osl = slice(ob * 512, (ob + 1) * 512)
                        tg, bf = oo_tags[(tb * 8 + ob) % 4]
                        o_ps3 = psA.tile([128, 512], F32, tag=tg, bufs=bf,
                                         name=f"oo_{tb}_{ob}")
                        # fp8 DoubleRow: per head pair one hi.hi instruction,
                        # plus one (lo,hi)x(hi,lo) cross instruction per head
                        for hp in (0, 2):
                            nc.tensor.matmul(
                                o_ps3[:, :],
                                lhsT=aout8[b][:, hp:hp + 2, 1, tsl],
                                rhs=wo_sb[:, hp:hp + 2, 0, osl],
                                perf_mode=DR, start=(hp == 0), stop=False)
                            nc.tensor.matmul(
                                o_ps3[:, :],
                                lhsT=aout8[b][:, hp, 0:2, tsl],
                                rhs=wo_sb[:, hp, 0:2, osl],
                                perf_mode=DR, start=False, stop=False)
                            nc.tensor.matmul(
                                o_ps3[:, :],
                                lhsT=aout8[b][:, hp + 1, 0:2, tsl],
                                rhs=wo_sb[:, hp + 1, 0:2, osl],
                                perf_mode=DR, start=False, stop=(hp == 2))
                        osb = p3t.tile([128, 512], F32, tag="osb", bufs=8)
                        if (tb * 8 + ob) % 2 == 0:
                            nc.scalar.copy(out=osb, in_=o_ps3[:, :])
                        else:
                            nc.vector.tensor_copy(out=osb, in_=o_ps3[:, :])
                        nc.sync.dma_start(out=out_d[tb, :, osl], in_=osb)
    nc.compile()
    return nc


def _rope_tables():
    freqs = np.einsum("i,j->ij", np.arange(MAX_POS),
                      1.0 / 10000 ** (np.arange(0, HD, 2) / HD)).astype("float32")
    emb = np.concatenate((freqs, freqs), axis=-1)  # [pos, HD]
    return np.sin(emb), np.cos(emb)


def _split8(x):
    """Split fp32 array into (hi, lo) fp8e4 planes, stacked on a new axis
    just before the last: [..., n] -> [..., 2, n] with j0=hi, j1=lo."""
    xc = np.clip(x, -240.0, 240.0)
    hi = xc.astype(NP_FP8)
    lo = (xc - hi.astype(np.float32)).astype(NP_FP8)
    return np.ascontiguousarray(np.stack((hi, lo), axis=-2))


def _prep_inputs(hidden_states, attention_mask, position_ids, Wq, Wk, Wv, Wo):
    hs = np.ascontiguousarray(np.asarray(hidden_states, dtype=np.float32))
    am = np.asarray(attention_mask, dtype=np.float32)
    pid = np.asarray(position_ids).astype(np.int64).reshape(-1)

    sin, cos = _rope_tables()
    sinT = np.ascontiguousarray(sin[pid].T)   # [HD, T]
    cosT = np.ascontiguousarray(cos[pid].T)
    sinT[0:HD // 2] *= -1.0                    # fold rotate-half sign
    sin_in = (sinT * (1.0 / WS)).astype(NP_BF16)   # undo weight prescale
    cos_in = (cosT * (1.0 / WS)).astype(NP_BF16)

    hsT = hs.reshape(T, HIDDEN).T                          # [HIDDEN, T]
    hsT = np.ascontiguousarray(
        hsT.reshape(KT, 128, T).transpose(1, 0, 2))        # [128, KT, T]
    # hidden hi/lo planes: [128, KT, 2, T]
    hh = hsT.astype(NP_FP8)
    hl = (hsT - hh.astype(np.float32)).astype(NP_FP8)
    hst_in = np.ascontiguousarray(np.stack((hh, hl), axis=2))

    # causal masks for diagonal blocks: allowed iff c >= 128*o + r
    r = np.arange(128)[:, None]
    c = np.arange(512)[None, :]
    masks = np.stack([np.where(c >= 128 * o + r, 0.0, NEG) for o in range(4)])
    masks = np.ascontiguousarray(masks.transpose(1, 0, 2)).astype(NP_BF16)  # [128,4,512]

    kb = np.where(am.reshape(-1) > 0, KB_SHIFT, NEG).astype(np.float32)
    kb_in = np.ascontiguousarray(kb.reshape(TT, 128).T)   # [128, TT]

    scale = 1.0 / math.sqrt(HD)
    Wq = np.asarray(Wq, dtype=np.float32) * (scale * WS)
    Wk = np.asarray(Wk, dtype=np.float32) * WS
    Wv = np.asarray(Wv, dtype=np.float32) * WS
    Wo = np.asarray(Wo, dtype=np.float32) * WS

    in_maps = []
    for m in range(NCORES):
        wq_m = np.ascontiguousarray(Wq[:, m * HPC * HD:(m + 1) * HPC * HD])
        wk_m = np.ascontiguousarray(Wk[:, m * HD:(m + 1) * HD])
        wv_m = np.ascontiguousarray(Wv[:, m * HD:(m + 1) * HD])
        wo_m = np.ascontiguousarray(Wo[m * HPC * HD:(m + 1) * HPC * HD, :])
        # per-kt (lo, hi) weight planes (cross instrs pair (lo,hi)x(hi,lo));
        # _split8 gives (hi, lo) so flip the plane axis
        wq8 = _split8(wq_m.reshape(KT, 128, HPC, HD).transpose(1, 2, 0, 3))
        wk8 = _split8(wk_m.reshape(KT, 128, HD).transpose(1, 0, 2))
        wv8 = _split8(wv_m.reshape(KT, 128, HD).transpose(1, 0, 2))
        wo8 = _split8(wo_m.reshape(HPC, 128, HIDDEN).transpose(1, 0, 2))
        in_maps.append({
            "hst": hst_in,
            "sint": sin_in,
            "cost": cos_in,
            # [128, HPC, KT, 2, HD]: partition-major, per-head blocked,
            # planes j0=lo j1=hi
            "wq": np.ascontiguousarray(wq8[:, :, :, ::-1, :]),
            "wk": np.ascontiguousarray(wk8[:, :, ::-1, :]),
            "wv": np.ascontiguousarray(wv8[:, :, ::-1, :]),
            # [128, HPC, 2, HIDDEN]: planes j0=hi j1=lo
            "wo": np.ascontiguousarray(wo8),
            "masks": masks,
            "kbias": kb_in,
        })
    return in_maps


def get_program():
    global _PROGRAM
    if _PROGRAM is None:
        _PROGRAM = _build_program()
    return _PROGRAM


def kernel(**inputs):
    nc = get_program()
    in_maps = _prep_inputs(
        inputs["hidden_states"], inputs["attention_mask"], inputs["position_ids"],
        inputs["Wq"], inputs["Wk"], inputs["Wv"], inputs["Wo"])
    res = run_bass_kernel_spmd(nc, in_maps, core_ids=list(range(NCORES)))
    acc = np.zeros((TT, 128, HIDDEN), dtype=np.float32)
    for r in res.results:
        acc += r["out"]
    return (acc * (1.0 / (SA * WS))).reshape(B, S, HIDDEN)


# revision 10
# speedup vs baseline: 1.1495x; 1.0067x over previous
"""Trainium2 Bass kernel for FlaxSapama (Llama-style) attention block.

Strategy: tensor-parallel over heads across 8 NeuronCores.
Core m owns Q heads [4m..4m+4) and KV head m (GQA group of 4), plus the
matching slice of Wo rows. Each core computes a full [T, HIDDEN] partial
output (its heads' contribution through Wo); the host sums the 8 partials.

Per-core pipeline:
  1. QKV projections in fp8e4 DoubleRow mode with hi/lo error
     compensation: X ~ X_hi + X_lo (both fp8), W.T@X ~ Whi.T@Xhi +
     Wlo.T@Xhi + Whi.T@Xlo (lo.lo dropped). The three plane-products per
     contraction tile pack into 1.5 DoubleRow instructions (2 planes each
     at 0.5 cycles/row), i.e. 0.75x the bf16 matmul cycles at ~bf16
     accuracy. Weights pre-split on host (scaled x1024 into fp8 range,
     compensated via sin/cos tables, V-copy scale, and host divide);
     hidden states pre-split on host. RoPE applied on PSUM evacuation.
  2. Attention with scores computed transposed in bf16: S^T[k,q] tiles;
     softmax denominators via ones-matmul accumulation in PSUM; causal
     masking via additive mask tiles on diagonal blocks only; exp on
     ScalarE with per-partition key-padding bias; 1/denominator broadcast
     across partitions via gpsimd.partition_broadcast. Normalized head
     outputs are written as fp8 hi/lo planes (x16, folded into the
     reciprocal via the ones value) for phase 3.
  3. Output projection in fp8e4 DoubleRow with hi/lo planes for both
     aout and Wo: per head-pair, one hi.hi instruction plus one cross
     instruction per head (0.75x bf16 cycles).

Tiles are split per (head, batch) so the Tile scheduler can overlap the
three phases across batches.
"""

import math

import numpy as np
import ml_dtypes

import concourse.bacc as bacc
import concourse.tile as tile
import concourse.mybir as mybir
from concourse.bass_utils import run_bass_kernel_spmd

BF16 = mybir.dt.bfloat16
F32 = mybir.dt.float32
FP8 = mybir.dt.float8e4
NP_BF16 = ml_dtypes.bfloat16
NP_FP8 = ml_dtypes.float8_e4m3
DR = mybir.MatmulPerfMode.DoubleRow

HIDDEN = 4096
N_HEADS = 32
N_KV = 8
HD = 128          # head dim
MAX_POS = 4096
B, S = 2, 2048
T = B * S         # 4096 tokens
NCORES = 8
HPC = N_HEADS // NCORES      # 4 q heads per core
KT = HIDDEN // 128           # 32 contraction tiles for projections
NB = T // 512                # 8 token blocks of 512
TT = T // 128                # 32 token tiles of 128
SKT = S // 128               # 16 k-pos tiles per batch
NEG = -1.0e9

WS = 1024.0       # weight prescale into fp8 range (exact power of 2)
SA = 16.0         # aout prescale (folded into ones value)
ONES_VAL = 1.0 / SA
KB_SHIFT = -4.0 * math.log(2.0)   # global exp shift (cancels in softmax)

_PROGRAM = None


def _build_program():
    nc = bacc.Bacc(None, target_bir_lowering=False)

    # all inputs partition-major so DMA runs are >=512B contiguous per
    # partition; fp8 tensors carry (hi, lo) planes for error compensation
    hst_d = nc.dram_tensor("hst", [128, KT, 2, T], FP8, kind="ExternalInput")
    sin_d = nc.dram_tensor("sint", [128, T], BF16, kind="ExternalInput")
    cos_d = nc.dram_tensor("cost", [128, T], BF16, kind="ExternalInput")
    wq_d = nc.dram_tensor("wq", [128, HPC, KT, 2, HD], FP8, kind="ExternalInput")
    wk_d = nc.dram_tensor("wk", [128, KT, 2, HD], FP8, kind="ExternalInput")
    wv_d = nc.dram_tensor("wv", [128, KT, 2, HD], FP8, kind="ExternalInput")
    wo_d = nc.dram_tensor("wo", [128, HPC, 2, HIDDEN], FP8, kind="ExternalInput")
    masks_d = nc.dram_tensor("masks", [128, 4, 512], BF16, kind="ExternalInput")
    kb_d = nc.dram_tensor("kbias", [128, TT], F32, kind="ExternalInput")
    out_d = nc.dram_tensor("out", [TT, 128, HIDDEN], F32, kind="ExternalOutput")

    with tile.TileContext(nc) as tc:
        # one PSUM pool for the whole kernel: a shared "big" tag lets
        # consecutive phases overlap through slot recycling instead of
        # serializing on pool region reuse; "den" gets the other banks
        with tc.tile_pool(name="qkv", bufs=1) as pool_qkv, \
             tc.tile_pool(name="psA", bufs=4, space="PSUM") as psA:
            # per-(head, batch) tiles so phases can overlap across batches
            qT = [[pool_qkv.tile([128, S], BF16, name=f"qT_{h}_{b}")
                   for b in range(B)] for h in range(HPC)]
            kTt = [pool_qkv.tile([128, S], BF16, name=f"kT_{b}") for b in range(B)]
            vt = [pool_qkv.tile([128, SKT, HD], BF16, name=f"v_{b}") for b in range(B)]
            # normalized head outputs as fp8 planes: per head j0=lo, j1=hi
            aout8 = [pool_qkv.tile([128, HPC, 2, S], FP8, name=f"ao8_{b}")
                     for b in range(B)]
            masks_sb = pool_qkv.tile([128, 4, 512], BF16)
            kb_sb = pool_qkv.tile([128, TT], F32)
            ones_sb = pool_qkv.tile([128, 1], BF16)
            nc.vector.memset(ones_sb, ONES_VAL)

            # ---------------- Phase 1: QKV projections + RoPE ----------------
            with tc.tile_pool(name="p1w", bufs=1) as p1w, \
                 tc.tile_pool(name="p1h", bufs=2) as p1h, \
                 tc.tile_pool(name="p1t", bufs=2) as p1t:

                def load_hst(nb):
                    tok = slice(nb * 512, (nb + 1) * 512)
                    tiles = [p1h.tile([128, KT // 4, 2, 512], FP8, tag=f"hst{q}",
                                      name=f"hst{q}_{nb}")
                             for q in range(4)]
                    for q in range(4):
                        nc.sync.dma_start(
                            out=tiles[q], in_=hst_d[:, q * 8:(q + 1) * 8, :, tok])
                    sin_t = p1h.tile([128, 512], BF16, tag="sin", name=f"sin_{nb}")
                    cos_t = p1h.tile([128, 512], BF16, tag="cos", name=f"cos_{nb}")
                    nc.sync.dma_start(out=sin_t, in_=sin_d[:, tok])
                    nc.sync.dma_start(out=cos_t, in_=cos_d[:, tok])
                    return tiles, sin_t, cos_t

                def qkv_matmuls(psum, w_of, hst_t, ktp, kt_lo, kt_hi):
                    # per kt pair: one hi.hi instruction (strided planes) +
                    # one cross instruction per kt ((lo,hi)x(hi,lo) planes)
                    q, r = ktp // 8, ktp % 8
                    ht = hst_t[q]
                    nc.tensor.matmul(psum[:, :],
                                     lhsT=w_of(slice(ktp, ktp + 2), 1),
                                     rhs=ht[:, r:r + 2, 0, :],
                                     perf_mode=DR,
                                     start=(ktp == kt_lo), stop=False)
                    nc.tensor.matmul(psum[:, :],
                                     lhsT=w_of(ktp, slice(0, 2)),
                                     rhs=ht[:, r, 0:2, :],
                                     perf_mode=DR, start=False, stop=False)
                    nc.tensor.matmul(psum[:, :],
                                     lhsT=w_of(ktp + 1, slice(0, 2)),
                                     rhs=ht[:, r + 1, 0:2, :],
                                     perf_mode=DR, start=False,
                                     stop=(ktp == kt_hi - 2))

                wq_sb = [p1w.tile([128, KT, 2, HD], FP8, name=f"wq_{h}")
                         for h in range(HPC)]
                wk_sb = p1w.tile([128, KT, 2, HD], FP8)
                wv_a = p1w.tile([128, 8, 2, HD], FP8)
                wv_b = p1w.tile([128, KT - 8, 2, HD], FP8)
                # DMA emission order matches first-block consumption; block-0
                # hst arrives as 2-kt pieces so matmul waits are fine-grained
                nc.sync.dma_start(out=wv_a, in_=wv_d[:, 0:8, :, :])
                h0 = [p1h.tile([128, KT // 4, 2, 512], FP8, tag=f"hst{q}",
                               name=f"hst{q}_0") for q in range(4)]
                nc.sync.dma_start(out=wk_sb, in_=wk_d[:, :, :, :])
                nc.sync.dma_start(out=wv_b, in_=wv_d[:, 8:KT, :, :])
                for kt2 in range(0, KT, 2):
                    nc.sync.dma_start(out=h0[kt2 // 8][:, kt2 % 8:kt2 % 8 + 2, :, :],
                                      in_=hst_d[:, kt2:kt2 + 2, :, 0:512])
                sin_0 = p1h.tile([128, 512], BF16, tag="sin", name="sin_0")
                cos_0 = p1h.tile([128, 512], BF16, tag="cos", name="cos_0")
                nc.sync.dma_start(out=sin_0, in_=sin_d[:, 0:512])
                nc.sync.dma_start(out=cos_0, in_=cos_d[:, 0:512])
                nb0_tiles = (h0, sin_0, cos_0)
                for h in range(HPC):
                    nc.sync.dma_start(out=wq_sb[h], in_=wq_d[:, h, :, :, :])
                nc.sync.dma_start(out=masks_sb, in_=masks_d[:, :, :])
                nc.sync.dma_start(out=kb_sb, in_=kb_d[:, :])

                def make_w_of(m):
                    if m < HPC:
                        return lambda sl3, sl2: wq_sb[m][:, sl3, sl2, :]
                    if m == HPC:
                        return lambda sl3, sl2: wk_sb[:, sl3, sl2, :]

                    def w_of(sl3, sl2):
                        st = sl3 if isinstance(sl3, int) else sl3.start
                        if st < 8:
                            return wv_a[:, sl3, sl2, :]
                        sl3b = (sl3 - 8 if isinstance(sl3, int)
                                else slice(sl3.start - 8, sl3.stop - 8))
                        return wv_b[:, sl3b, sl2, :]
                    return w_of

                def evac(nb, m, psum, sin_t, cos_t):
                    b, qb = nb // 4, nb % 4
                    bsl = slice(qb * 512, (qb + 1) * 512)
                    if m <= HPC:
                        # rope: out = x*cos + shift_half(x)*sin' (sign in
                        # sin'; 1/WS folded into the sin/cos tables).
                        # cross-half reads straight from PSUM (SB-SB ops
                        # need equal base partitions; PSUM+SB is exempt)
                        tmp = p1t.tile([128, 512], BF16, tag="tmp")
                        nc.vector.tensor_mul(tmp[0:64, :], psum[64:128, :], sin_t[0:64, :])
                        nc.vector.tensor_mul(tmp[64:128, :], psum[0:64, :], sin_t[64:128, :])
                        t2 = p1t.tile([128, 512], BF16, tag="t2")
                        nc.vector.tensor_mul(t2, psum[:, :], cos_t)
                        dest = qT[m][b][:, bsl] if m < HPC else kTt[b][:, bsl]
                        nc.vector.tensor_add(dest, t2, tmp)
                    else:
                        v_bf = p1t.tile([128, 512], BF16, tag="vbf")
                        nc.scalar.mul(v_bf, psum[:, :], 1.0 / WS)
                        for j in range(4):
                            nc.sync.dma_start_transpose(
                                out=vt[b][:, qb * 4 + j, :],
                                in_=v_bf[:, j * 128:(j + 1) * 128])

                # block 0: quarter-major across all 6 chains so PE work per
                # arriving hst piece is 6x a single chain's (hides the cold
                # DMA); 6 concurrent psums borrow the attention-phase tags
                nb0_psums = []
                for m, (tg, bf) in enumerate((("big", 3), ("big", 3), ("big", 3),
                                              ("obank", 3), ("obank", 3),
                                              ("den", 2))):
                    nb0_psums.append(psA.tile([128, 512], F32, tag=tg, bufs=bf,
                                              name=f"pj0_{m}"))
                h0_t, sin_0, cos_0 = nb0_tiles
                for q4 in range(4):
                    for m in (5, 4, 0, 1, 2, 3):
                        for ktp in range(q4 * 8, q4 * 8 + 8, 2):
                            qkv_matmuls(nb0_psums[m], make_w_of(m), h0_t,
                                        ktp, 0, KT)
                for m in (5, 4, 0, 1, 2, 3):
                    evac(0, m, nb0_psums[m], sin_0, cos_0)

                for nb in range(1, NB):
                    hst_t, sin_t, cos_t = load_hst(nb)
                    for m in ((5, 4, 0, 1, 2, 3) if nb < NB - 1 else (0, 1, 2, 3, 4, 5)):
                        psum = psA.tile([128, 512], F32, tag="big", bufs=3,
                                        name=f"pj_{nb}_{m}")
                        for ktp in range(0, KT, 2):
                            qkv_matmuls(psum, make_w_of(m), hst_t, ktp, 0, KT)
                        evac(nb, m, psum, sin_t, cos_t)

            # ---------------- Phases 2+3 ----------------
            # p3 pools open (and wo loads) before p2 pools so the wo DMA only
            # waits on phase-1 readers, not on all of phase 2
            with tc.tile_pool(name="p3c", bufs=1) as p3c, \
                 tc.tile_pool(name="p3t", bufs=4) as p3t, \
                 tc.tile_pool(name="p2c", bufs=1) as p2c, \
                 tc.tile_pool(name="p2t", bufs=4) as p2t:
                wo_sb = p3c.tile([128, HPC, 2, HIDDEN], FP8)
                for h in range(HPC):
                    nc.sync.dma_start(out=wo_sb[:, h, :, :], in_=wo_d[:, h, :, :])
                def emit_s(b, h, qb, kt):
                    # diagonal tiles (kt-4qb = o > 0): columns < 128*o are
                    # fully causal-masked -> skip them entirely
                    o_off = kt - 4 * qb
                    c0 = 128 * o_off if o_off > 0 else 0
                    cs = slice(c0, 512)
                    s_ps = psA.tile([128, 512], F32, tag="big", bufs=3,
                                    name=f"s_{b}_{h}_{qb}_{kt}")
                    nc.tensor.matmul(
                        s_ps[:, cs],
                        lhsT=kTt[b][:, kt * 128:(kt + 1) * 128],
                        rhs=qT[h][b][:, qb * 512 + c0:(qb + 1) * 512],
                        start=True, stop=True)
                    if o_off >= 0:
                        # only the 128-col boundary chunk holds the causal
                        # staircase; columns >= c0+128 are fully valid (mask 0)
                        cm = slice(c0, c0 + 128)
                        nc.vector.tensor_add(s_ps[:, cm], s_ps[:, cm],
                                             masks_sb[:, o_off, cm])
                    p_bf = p2t.tile([128, 512], BF16, tag="p", bufs=10)
                    gk = b * SKT + kt
                    nc.scalar.activation(
                        out=p_bf[:, cs], in_=s_ps[:, cs],
                        func=mybir.ActivationFunctionType.Exp,
                        bias=kb_sb[:, gk:gk + 1], scale=1.0)
                    return b, h, qb, kt, p_bf, cs

                acc = {}
                tails = []

                def emit_dp(b, h, qb, kt, p_bf, cs):
                    nkt = 4 * (qb + 1)
                    if kt == 0:
                        acc[(b, h, qb)] = (
                            psA.tile([128, 512], F32, tag="obank", bufs=3,
                                     name=f"o_{b}_{h}_{qb}"),
                            psA.tile([1, 512], F32, tag="den", bufs=2,
                                     name=f"den_{b}_{h}_{qb}"))
                    o_ps, den_ps = acc[(b, h, qb)]
                    nc.tensor.matmul(den_ps[:, cs], lhsT=ones_sb[:, :],
                                     rhs=p_bf[:, cs],
                                     start=(kt == 0), stop=(kt == nkt - 1))
                    nc.tensor.matmul(o_ps[:, cs], lhsT=vt[b][:, kt, :],
                                     rhs=p_bf[:, cs],
                                     start=(kt == 0), stop=(kt == nkt - 1))
                    if kt == nkt - 1:
                        tails.append((b, h, qb))
                    elif kt == 2 and tails:
                        # deferred: run the previous qb's normalization on DVE
                        # after this qb's first mask-adds, not before them
                        emit_tail(*tails.pop(0))

                def emit_tail(b, h, qb):
                    o_ps, den_ps = acc.pop((b, h, qb))
                    qsl = slice(qb * 512, (qb + 1) * 512)
                    recip = p2t.tile([1, 512], F32, tag="recip")
                    nc.vector.reciprocal(recip, den_ps[:, :])
                    rb = p2t.tile([128, 512], F32, tag="rb")
                    nc.gpsimd.partition_broadcast(rb[:, :], recip[:, :])
                    nrm = p2t.tile([128, 512], BF16, tag="nrm")
                    nc.vector.tensor_mul(nrm, o_ps[:, :], rb[:, :])
                    # fp8 hi/lo planes of the (x SA) normalized output
                    hi = aout8[b][:, h, 1, qsl]
                    nc.scalar.copy(out=hi, in_=nrm)
                    nc.vector.tensor_sub(aout8[b][:, h, 0, qsl], nrm, hi)

                # ---------------- Phase 3 chain emitter ----------------
                oo_tags = (("big", 3), ("big", 3), ("obank", 3), ("den", 2))
                oo_idx = [0]

                def emit_oo(tb, ob, tags=None):
                    b, tloc = tb // SKT, tb % SKT
                    tsl = slice(tloc * 128, (tloc + 1) * 128)
                    osl = slice(ob * 512, (ob + 1) * 512)
                    i = oo_idx[0]
                    oo_idx[0] += 1
                    tg, bf = (tags or oo_tags)[i % len(tags or oo_tags)]
                    o_ps3 = psA.tile([128, 512], F32, tag=tg, bufs=bf,
                                     name=f"oo_{tb}_{ob}")
                    # fp8 DoubleRow: per head pair one hi.hi instruction,
                    # plus one (lo,hi)x(hi,lo) cross instruction per head
                    for hp in (0, 2):
                        nc.tensor.matmul(
                            o_ps3[:, :],
                            lhsT=aout8[b][:, hp:hp + 2, 1, tsl],
                            rhs=wo_sb[:, hp:hp + 2, 0, osl],
                            perf_mode=DR, start=(hp == 0), stop=False)
                        nc.tensor.matmul(
                            o_ps3[:, :],
                            lhsT=aout8[b][:, hp, 0:2, tsl],
                            rhs=wo_sb[:, hp, 0:2, osl],
                            perf_mode=DR, start=False, stop=False)
                        nc.tensor.matmul(
                            o_ps3[:, :],
                            lhsT=aout8[b][:, hp + 1, 0:2, tsl],
                            rhs=wo_sb[:, hp + 1, 0:2, osl],
                            perf_mode=DR, start=False, stop=(hp == 2))
                    osb = p3t.tile([128, 512], F32, tag="osb", bufs=8)
                    if i % 2 == 0:
                        nc.scalar.copy(out=osb, in_=o_ps3[:, :])
                    else:
                        nc.vector.tensor_copy(out=osb, in_=o_ps3[:, :])
                    nc.sync.dma_start(out=out_d[tb, :, osl], in_=osb)

                # one software pipeline across the whole attention phase:
                # den/pv (and each qb's normalization tail) lag the scores
                # matmul by 5 iterations so PE never waits on the exp (ACT).
                # During batch-1 attention, weave in phase-3 chains of batch 0
                # to keep PE fed through ACT-bound stretches.
                oo_work = [(tb, ob) for tb in range(TT) for ob in range(8)]
                woven = 0
                pend = []
                for b in range(B):
                    for h in range(HPC):
                        for qb in range(4):
                            for kt in range(4 * (qb + 1)):
                                pend.append(emit_s(b, h, qb, kt))
                                if len(pend) > 7:
                                    emit_dp(*pend.pop(0))
                            if b == 1:
                                # woven chains avoid the "den" tag: both den
                                # slots are held by live accumulators here
                                for _ in range(3):
                                    emit_oo(*oo_work[woven],
                                            tags=(("big", 3), ("obank", 3)))
                                    woven += 1
                for args in pend:
                    emit_dp(*args)
                while tails:
                    emit_tail(*tails.pop(0))

            # ---------------- Phase 3: output projection (remainder) -------
                for tb, ob in oo_work[woven:]:
                    emit_oo(tb, ob)
    nc.compile()
    return nc


def _rope_tables():
    freqs = np.einsum("i,j->ij", np.arange(MAX_POS),
                      1.0 / 10000 ** (np.arange(0, HD, 2) / HD)).astype("float32")
    emb = np.concatenate((freqs, freqs), axis=-1)  # [pos, HD]
    return np.sin(emb), np.cos(emb)


def _split8(x):
    """Split fp32 array into (hi, lo) fp8e4 planes, stacked on a new axis
    just before the last: [..., n] -> [..., 2, n] with j0=hi, j1=lo."""
    xc = np.clip(x, -240.0, 240.0)
    hi = xc.astype(NP_FP8)
    lo = (xc - hi.astype(np.float32)).astype(NP_FP8)
    return np.ascontiguousarray(np.stack((hi, lo), axis=-2))


def _prep_inputs(hidden_states, attention_mask, position_ids, Wq, Wk, Wv, Wo):
    hs = np.ascontiguousarray(np.asarray(hidden_states, dtype=np.float32))
    am = np.asarray(attention_mask, dtype=np.float32)
    pid = np.asarray(position_ids).astype(np.int64).reshape(-1)

    sin, cos = _rope_tables()
    sinT = np.ascontiguousarray(sin[pid].T)   # [HD, T]
    cosT = np.ascontiguousarray(cos[pid].T)
    sinT[0:HD // 2] *= -1.0                    # fold rotate-half sign
    sin_in = (sinT * (1.0 / WS)).astype(NP_BF16)   # undo weight prescale
    cos_in = (cosT * (1.0 / WS)).astype(NP_BF16)

    hsT = hs.reshape(T, HIDDEN).T                          # [HIDDEN, T]
    hsT = np.ascontiguousarray(
        hsT.reshape(KT, 128, T).transpose(1, 0, 2))        # [128, KT, T]
    # hidden hi/lo planes: [128, KT, 2, T]
    hh = hsT.astype(NP_FP8)
    hl = (hsT - hh.astype(np.float32)).astype(NP_FP8)
    hst_in = np.ascontiguousarray(np.stack((hh, hl), axis=2))

    # causal masks for diagonal blocks: allowed iff c >= 128*o + r
    r = np.arange(128)[:, None]
    c = np.arange(512)[None, :]
    masks = np.stack([np.where(c >= 128 * o + r, 0.0, NEG) for o in range(4)])
    masks = np.ascontiguousarray(masks.transpose(1, 0, 2)).astype(NP_BF16)  # [128,4,512]

    kb = np.where(am.reshape(-1) > 0, KB_SHIFT, NEG).astype(np.float32)
    kb_in = np.ascontiguousarray(kb.reshape(TT, 128).T)   # [128, TT]

    scale = 1.0 / math.sqrt(HD)
    Wq = np.asarray(Wq, dtype=np.float32) * (scale * WS)
    Wk = np.asarray(Wk, dtype=np.float32) * WS
    Wv = np.asarray(Wv, dtype=np.float32) * WS
    Wo = np.asarray(Wo, dtype=np.float32) * WS

    in_maps = []
    for m in range(NCORES):
        wq_m = np.ascontiguousarray(Wq[:, m * HPC * HD:(m + 1) * HPC * HD])
        wk_m = np.ascontiguousarray(Wk[:, m * HD:(m + 1) * HD])
        wv_m = np.ascontiguousarray(Wv[:, m * HD:(m + 1) * HD])
        wo_m = np.ascontiguousarray(Wo[m * HPC * HD:(m + 1) * HPC * HD, :])
        # per-kt (lo, hi) weight planes (cross instrs pair (lo,hi)x(hi,lo));
        # _split8 gives (hi, lo) so flip the plane axis
        wq8 = _split8(wq_m.reshape(KT, 128, HPC, HD).transpose(1, 2, 0, 3))
        wk8 = _split8(wk_m.reshape(KT, 128, HD).transpose(1, 0, 2))
        wv8 = _split8(wv_m.reshape(KT, 128, HD).transpose(1, 0, 2))
        wo8 = _split8(wo_m.reshape(HPC, 128, HIDDEN).transpose(1, 0, 2))
        in_maps.append({
            "hst": hst_in,
            "sint": sin_in,
            "cost": cos_in,
            # [128, HPC, KT, 2, HD]: partition-major, per-head blocked,
            # planes j0=lo j1=hi
            "wq": np.ascontiguousarray(wq8[:, :, :, ::-1, :]),
            "wk": np.ascontiguousarray(wk8[:, :, ::-1, :]),
            "wv": np.ascontiguousarray(wv8[:, :, ::-1, :]),
            # [128, HPC, 2, HIDDEN]: planes j0=hi j1=lo
            "wo": np.ascontiguousarray(wo8),
            "masks": masks,
            "kbias": kb_in,
        })
    return in_maps


def get_program():
    global _PROGRAM
    if _PROGRAM is None:
        _PROGRAM = _build_program()
    return _PROGRAM


def kernel(**inputs):
    nc = get_program()
    in_maps = _prep_inputs(
        inputs["hidden_states"], inputs["attention_mask"], inputs["position_ids"],
        inputs["Wq"], inputs["Wk"], inputs["Wv"], inputs["Wo"])
    res = run_bass_kernel_spmd(nc, in_maps, core_ids=list(range(NCORES)))
    acc = np.zeros((TT, 128, HIDDEN), dtype=np.float32)
    for r in res.results:
        acc += r["out"]
    return (acc * (1.0 / (SA * WS))).reshape(B, S, HIDDEN)


# revision 11
# speedup vs baseline: 1.1673x; 1.0154x over previous
"""Trainium2 Bass kernel for FlaxSapama (Llama-style) attention block.

Strategy: tensor-parallel over heads across 8 NeuronCores.
Core m owns Q heads [4m..4m+4) and KV head m (GQA group of 4), plus the
matching slice of Wo rows. Each core computes a full [T, HIDDEN] partial
output (its heads' contribution through Wo); the host sums the 8 partials.

Per-core pipeline:
  1. QKV projections in fp8e4 DoubleRow mode with hi/lo error
     compensation: X ~ X_hi + X_lo (both fp8), W.T@X ~ Whi.T@Xhi +
     Wlo.T@Xhi + Whi.T@Xlo (lo.lo dropped). The three plane-products per
     contraction tile pack into 1.5 DoubleRow instructions (2 planes each
     at 0.5 cycles/row), i.e. 0.75x the bf16 matmul cycles at ~bf16
     accuracy. Weights pre-split on host (scaled x1024 into fp8 range,
     compensated via sin/cos tables, V-copy scale, and host divide);
     hidden states pre-split on host. RoPE applied on PSUM evacuation.
  2. Attention with scores computed transposed in bf16: S^T[k,q] tiles;
     softmax denominators via ones-matmul accumulation in PSUM; causal
     masking via additive mask tiles on diagonal blocks only; exp on
     ScalarE with per-partition key-padding bias; 1/denominator broadcast
     across partitions via gpsimd.partition_broadcast. Normalized head
     outputs are written as fp8 hi/lo planes (x16, folded into the
     reciprocal via the ones value) for phase 3.
  3. Output projection in fp8e4 DoubleRow with hi/lo planes for both
     aout and Wo: per head-pair, one hi.hi instruction plus one cross
     instruction per head (0.75x bf16 cycles).

Tiles are split per (head, batch) so the Tile scheduler can overlap the
three phases across batches.
"""

import math

import numpy as np
import ml_dtypes

import concourse.bacc as bacc
import concourse.tile as tile
import concourse.mybir as mybir
from concourse.bass_utils import run_bass_kernel_spmd

BF16 = mybir.dt.bfloat16
F32 = mybir.dt.float32
FP8 = mybir.dt.float8e4
NP_BF16 = ml_dtypes.bfloat16
NP_FP8 = ml_dtypes.float8_e4m3
DR = mybir.MatmulPerfMode.DoubleRow

HIDDEN = 4096
N_HEADS = 32
N_KV = 8
HD = 128          # head dim
MAX_POS = 4096
B, S = 2, 2048
T = B * S         # 4096 tokens
NCORES = 8
HPC = N_HEADS // NCORES      # 4 q heads per core
KT = HIDDEN // 128           # 32 contraction tiles for projections
NB = T // 512                # 8 token blocks of 512
TT = T // 128                # 32 token tiles of 128
SKT = S // 128               # 16 k-pos tiles per batch
NEG = -1.0e9

WS = 1024.0       # weight prescale into fp8 range (exact power of 2)
SA = 16.0         # aout prescale (folded into ones value)
ONES_VAL = 1.0 / SA
KB_SHIFT = -4.0 * math.log(2.0)   # global exp shift (cancels in softmax)

_PROGRAM = None


def _build_program():
    nc = bacc.Bacc(None, target_bir_lowering=False)

    # all inputs partition-major so DMA runs are >=512B contiguous per
    # partition; fp8 tensors carry (hi, lo) planes for error compensation
    hst_d = nc.dram_tensor("hst", [128, KT, 2, T], FP8, kind="ExternalInput")
    sin_d = nc.dram_tensor("sint", [128, T], BF16, kind="ExternalInput")
    cos_d = nc.dram_tensor("cost", [128, T], BF16, kind="ExternalInput")
    wq_d = nc.dram_tensor("wq", [128, HPC, KT, 2, HD], FP8, kind="ExternalInput")
    wk_d = nc.dram_tensor("wk", [128, KT, 2, HD], FP8, kind="ExternalInput")
    wv_d = nc.dram_tensor("wv", [128, KT, 2, HD], FP8, kind="ExternalInput")
    wo_d = nc.dram_tensor("wo", [128, HPC, 2, HIDDEN], FP8, kind="ExternalInput")
    masks_d = nc.dram_tensor("masks", [128, 4, 512], BF16, kind="ExternalInput")
    kb_d = nc.dram_tensor("kbias", [128, TT], F32, kind="ExternalInput")
    out_d = nc.dram_tensor("out", [TT, 128, HIDDEN], F32, kind="ExternalOutput")

    with tile.TileContext(nc) as tc:
        # one PSUM pool for the whole kernel: a shared "big" tag lets
        # consecutive phases overlap through slot recycling instead of
        # serializing on pool region reuse; "den" gets the other banks
        with tc.tile_pool(name="qkv", bufs=1) as pool_qkv, \
             tc.tile_pool(name="psA", bufs=4, space="PSUM") as psA:
            # per-(head, batch) tiles so phases can overlap across batches
            qT = [[pool_qkv.tile([128, S], BF16, name=f"qT_{h}_{b}")
                   for b in range(B)] for h in range(HPC)]
            kTt = [pool_qkv.tile([128, S], BF16, name=f"kT_{b}") for b in range(B)]
            vt = [pool_qkv.tile([128, SKT, HD], BF16, name=f"v_{b}") for b in range(B)]
            # normalized head outputs as fp8 planes: per head j0=lo, j1=hi
            aout8 = [pool_qkv.tile([128, HPC, 2, S], FP8, name=f"ao8_{b}")
                     for b in range(B)]
            masks_sb = pool_qkv.tile([128, 4, 512], BF16)
            kb_sb = pool_qkv.tile([128, TT], F32)
            ones_sb = pool_qkv.tile([128, 1], BF16)
            nc.vector.memset(ones_sb, ONES_VAL)

            # ---------------- Phase 1: QKV projections + RoPE ----------------
            with tc.tile_pool(name="p1w", bufs=1) as p1w, \
                 tc.tile_pool(name="p1h", bufs=2) as p1h, \
                 tc.tile_pool(name="p1t", bufs=2) as p1t:

                def load_hst(nb):
                    tok = slice(nb * 512, (nb + 1) * 512)
                    tiles = [p1h.tile([128, KT // 4, 2, 512], FP8, tag=f"hst{q}",
                                      name=f"hst{q}_{nb}")
                             for q in range(4)]
                    for q in range(4):
                        nc.sync.dma_start(
                            out=tiles[q], in_=hst_d[:, q * 8:(q + 1) * 8, :, tok])
                    sin_t = p1h.tile([128, 512], BF16, tag="sin", name=f"sin_{nb}")
                    cos_t = p1h.tile([128, 512], BF16, tag="cos", name=f"cos_{nb}")
                    nc.sync.dma_start(out=sin_t, in_=sin_d[:, tok])
                    nc.sync.dma_start(out=cos_t, in_=cos_d[:, tok])
                    return tiles, sin_t, cos_t

                def qkv_matmuls(psum, w_of, hst_t, ktp, kt_lo, kt_hi):
                    # per kt pair: one hi.hi instruction (strided planes) +
                    # one cross instruction per kt ((lo,hi)x(hi,lo) planes)
                    q, r = ktp // 8, ktp % 8
                    ht = hst_t[q]
                    nc.tensor.matmul(psum[:, :],
                                     lhsT=w_of(slice(ktp, ktp + 2), 1),
                                     rhs=ht[:, r:r + 2, 0, :],
                                     perf_mode=DR,
                                     start=(ktp == kt_lo), stop=False)
                    nc.tensor.matmul(psum[:, :],
                                     lhsT=w_of(ktp, slice(0, 2)),
                                     rhs=ht[:, r, 0:2, :],
                                     perf_mode=DR, start=False, stop=False)
                    nc.tensor.matmul(psum[:, :],
                                     lhsT=w_of(ktp + 1, slice(0, 2)),
                                     rhs=ht[:, r + 1, 0:2, :],
                                     perf_mode=DR, start=False,
                                     stop=(ktp == kt_hi - 2))

                wq_sb = [p1w.tile([128, KT, 2, HD], FP8, name=f"wq_{h}")
                         for h in range(HPC)]
                wk_sb = p1w.tile([128, KT, 2, HD], FP8)
                wv_a = p1w.tile([128, 8, 2, HD], FP8)
                wv_b = p1w.tile([128, KT - 8, 2, HD], FP8)
                # DMA emission order tracks block-0 quarter-major consumption
                # (the DMA device serializes transfers); block-0 hst arrives
                # as 2-kt pieces so matmul waits are fine-grained
                h0 = [p1h.tile([128, KT // 4, 2, 512], FP8, tag=f"hst{q}",
                               name=f"hst{q}_0") for q in range(4)]

                def h0_pieces(q):
                    for kt2 in range(q * 8, q * 8 + 8, 2):
                        nc.sync.dma_start(
                            out=h0[q][:, kt2 % 8:kt2 % 8 + 2, :, :],
                            in_=hst_d[:, kt2:kt2 + 2, :, 0:512])
                nc.sync.dma_start(out=wv_a, in_=wv_d[:, 0:8, :, :])
                h0_pieces(0)
                nc.sync.dma_start(out=wk_sb, in_=wk_d[:, :, :, :])
                for h in range(HPC):
                    nc.sync.dma_start(out=wq_sb[h], in_=wq_d[:, h, :, :, :])
                nc.sync.dma_start(out=wv_b, in_=wv_d[:, 8:KT, :, :])
                for q in (1, 2, 3):
                    h0_pieces(q)
                sin_0 = p1h.tile([128, 512], BF16, tag="sin", name="sin_0")
                cos_0 = p1h.tile([128, 512], BF16, tag="cos", name="cos_0")
                nc.sync.dma_start(out=sin_0, in_=sin_d[:, 0:512])
                nc.sync.dma_start(out=cos_0, in_=cos_d[:, 0:512])
                nb0_tiles = (h0, sin_0, cos_0)
                nc.sync.dma_start(out=masks_sb, in_=masks_d[:, :, :])
                nc.sync.dma_start(out=kb_sb, in_=kb_d[:, :])

                def make_w_of(m):
                    if m < HPC:
                        return lambda sl3, sl2: wq_sb[m][:, sl3, sl2, :]
                    if m == HPC:
                        return lambda sl3, sl2: wk_sb[:, sl3, sl2, :]

                    def w_of(sl3, sl2):
                        st = sl3 if isinstance(sl3, int) else sl3.start
                        if st < 8:
                            return wv_a[:, sl3, sl2, :]
                        sl3b = (sl3 - 8 if isinstance(sl3, int)
                                else slice(sl3.start - 8, sl3.stop - 8))
                        return wv_b[:, sl3b, sl2, :]
                    return w_of

                def evac(nb, m, psum, sin_t, cos_t):
                    b, qb = nb // 4, nb % 4
                    bsl = slice(qb * 512, (qb + 1) * 512)
                    if m <= HPC:
                        # rope: out = x*cos + shift_half(x)*sin' (sign in
                        # sin'; 1/WS folded into the sin/cos tables).
                        # cross-half reads straight from PSUM (SB-SB ops
                        # need equal base partitions; PSUM+SB is exempt)
                        tmp = p1t.tile([128, 512], BF16, tag="tmp")
                        nc.vector.tensor_mul(tmp[0:64, :], psum[64:128, :], sin_t[0:64, :])
                        nc.vector.tensor_mul(tmp[64:128, :], psum[0:64, :], sin_t[64:128, :])
                        t2 = p1t.tile([128, 512], BF16, tag="t2")
                        nc.vector.tensor_mul(t2, psum[:, :], cos_t)
                        dest = qT[m][b][:, bsl] if m < HPC else kTt[b][:, bsl]
                        nc.vector.tensor_add(dest, t2, tmp)
                    else:
                        v_bf = p1t.tile([128, 512], BF16, tag="vbf")
                        nc.scalar.mul(v_bf, psum[:, :], 1.0 / WS)
                        for j in range(4):
                            nc.sync.dma_start_transpose(
                                out=vt[b][:, qb * 4 + j, :],
                                in_=v_bf[:, j * 128:(j + 1) * 128])

                # block 0: quarter-major across all 6 chains so PE work per
                # arriving hst piece is 6x a single chain's (hides the cold
                # DMA); 6 concurrent psums borrow the attention-phase tags
                nb0_psums = []
                for m, (tg, bf) in enumerate((("big", 4), ("big", 4), ("big", 4),
                                              ("big", 4), ("obank", 2),
                                              ("obank", 2))):
                    nb0_psums.append(psA.tile([128, 512], F32, tag=tg, bufs=bf,
                                              name=f"pj0_{m}"))
                h0_t, sin_0, cos_0 = nb0_tiles
                for q4 in range(4):
                    for m in (5, 4, 0, 1, 2, 3):
                        for ktp in range(q4 * 8, q4 * 8 + 8, 2):
                            qkv_matmuls(nb0_psums[m], make_w_of(m), h0_t,
                                        ktp, 0, KT)
                for m in (5, 4, 0, 1, 2, 3):
                    evac(0, m, nb0_psums[m], sin_0, cos_0)

                for nb in range(1, NB):
                    hst_t, sin_t, cos_t = load_hst(nb)
                    for m in ((5, 4, 0, 1, 2, 3) if nb < NB - 1 else (0, 1, 2, 3, 4, 5)):
                        psum = psA.tile([128, 512], F32, tag="big", bufs=4,
                                        name=f"pj_{nb}_{m}")
                        for ktp in range(0, KT, 2):
                            qkv_matmuls(psum, make_w_of(m), hst_t, ktp, 0, KT)
                        evac(nb, m, psum, sin_t, cos_t)

            # ---------------- Phases 2+3 ----------------
            # p3 pools open (and wo loads) before p2 pools so the wo DMA only
            # waits on phase-1 readers, not on all of phase 2
            with tc.tile_pool(name="p3c", bufs=1) as p3c, \
                 tc.tile_pool(name="p3t", bufs=4) as p3t, \
                 tc.tile_pool(name="p2c", bufs=1) as p2c, \
                 tc.tile_pool(name="p2t", bufs=4) as p2t:
                wo_sb = p3c.tile([128, HPC, 2, HIDDEN], FP8)
                for h in range(HPC):
                    nc.sync.dma_start(out=wo_sb[:, h, :, :], in_=wo_d[:, h, :, :])
                def emit_s(b, h, qb, kt):
                    # diagonal tiles (kt-4qb = o > 0): columns < 128*o are
                    # fully causal-masked -> skip them entirely
                    o_off = kt - 4 * qb
                    c0 = 128 * o_off if o_off > 0 else 0
                    cs = slice(c0, 512)
                    s_ps = psA.tile([128, 512], F32, tag="big", bufs=4,
                                    name=f"s_{b}_{h}_{qb}_{kt}")
                    nc.tensor.matmul(
                        s_ps[:, cs],
                        lhsT=kTt[b][:, kt * 128:(kt + 1) * 128],
                        rhs=qT[h][b][:, qb * 512 + c0:(qb + 1) * 512],
                        start=True, stop=True)
                    if o_off >= 0:
                        # only the 128-col boundary chunk holds the causal
                        # staircase; columns >= c0+128 are fully valid (mask 0)
                        cm = slice(c0, c0 + 128)
                        nc.vector.tensor_add(s_ps[:, cm], s_ps[:, cm],
                                             masks_sb[:, o_off, cm])
                    p_bf = p2t.tile([128, 512], BF16, tag="p", bufs=10)
                    gk = b * SKT + kt
                    nc.scalar.activation(
                        out=p_bf[:, cs], in_=s_ps[:, cs],
                        func=mybir.ActivationFunctionType.Exp,
                        bias=kb_sb[:, gk:gk + 1], scale=1.0)
                    return b, h, qb, kt, p_bf, cs

                acc = {}
                tails = []

                def emit_dp(b, h, qb, kt, p_bf, cs):
                    nkt = 4 * (qb + 1)
                    if kt == 0:
                        acc[(b, h, qb)] = (
                            psA.tile([128, 512], F32, tag="obank", bufs=2,
                                     name=f"o_{b}_{h}_{qb}"),
                            psA.tile([1, 512], F32, tag="den", bufs=2,
                                     name=f"den_{b}_{h}_{qb}"))
                    o_ps, den_ps = acc[(b, h, qb)]
                    nc.tensor.matmul(den_ps[:, cs], lhsT=ones_sb[:, :],
                                     rhs=p_bf[:, cs],
                                     start=(kt == 0), stop=(kt == nkt - 1))
                    nc.tensor.matmul(o_ps[:, cs], lhsT=vt[b][:, kt, :],
                                     rhs=p_bf[:, cs],
                                     start=(kt == 0), stop=(kt == nkt - 1))
                    if kt == nkt - 1:
                        tails.append((b, h, qb))
                    elif kt == 2 and tails:
                        # deferred: run the previous qb's normalization on DVE
                        # after this qb's first mask-adds, not before them
                        emit_tail(*tails.pop(0))

                def emit_tail(b, h, qb):
                    o_ps, den_ps = acc.pop((b, h, qb))
                    qsl = slice(qb * 512, (qb + 1) * 512)
                    recip = p2t.tile([1, 512], F32, tag="recip")
                    nc.vector.reciprocal(recip, den_ps[:, :])
                    rb = p2t.tile([128, 512], F32, tag="rb")
                    nc.gpsimd.partition_broadcast(rb[:, :], recip[:, :])
                    nrm = p2t.tile([128, 512], BF16, tag="nrm")
                    nc.vector.tensor_mul(nrm, o_ps[:, :], rb[:, :])
                    # fp8 hi/lo planes of the (x SA) normalized output
                    hi = aout8[b][:, h, 1, qsl]
                    nc.gpsimd.tensor_copy(out=hi, in_=nrm)
                    nc.vector.tensor_sub(aout8[b][:, h, 0, qsl], nrm, hi)

                # ---------------- Phase 3 chain emitter ----------------
                oo_tags = (("big", 4), ("big", 4), ("obank", 2), ("den", 2))
                oo_idx = [0]

                def emit_oo(tb, ob, tags=None, osb_dve=False):
                    b, tloc = tb // SKT, tb % SKT
                    tsl = slice(tloc * 128, (tloc + 1) * 128)
                    osl = slice(ob * 512, (ob + 1) * 512)
                    i = oo_idx[0]
                    oo_idx[0] += 1
                    tg, bf = (tags or oo_tags)[i % len(tags or oo_tags)]
                    o_ps3 = psA.tile([128, 512], F32, tag=tg, bufs=bf,
                                     name=f"oo_{tb}_{ob}")
                    # fp8 DoubleRow: per head pair one hi.hi instruction,
                    # plus one (lo,hi)x(hi,lo) cross instruction per head
                    for hp in (0, 2):
                        nc.tensor.matmul(
                            o_ps3[:, :],
                            lhsT=aout8[b][:, hp:hp + 2, 1, tsl],
                            rhs=wo_sb[:, hp:hp + 2, 0, osl],
                            perf_mode=DR, start=(hp == 0), stop=False)
                        nc.tensor.matmul(
                            o_ps3[:, :],
                            lhsT=aout8[b][:, hp, 0:2, tsl],
                            rhs=wo_sb[:, hp, 0:2, osl],
                            perf_mode=DR, start=False, stop=False)
                        nc.tensor.matmul(
                            o_ps3[:, :],
                            lhsT=aout8[b][:, hp + 1, 0:2, tsl],
                            rhs=wo_sb[:, hp + 1, 0:2, osl],
                            perf_mode=DR, start=False, stop=(hp == 2))
                    osb = p3t.tile([128, 512], F32, tag="osb", bufs=8)
                    if not osb_dve and i % 2 == 0:
                        nc.scalar.copy(out=osb, in_=o_ps3[:, :])
                    else:
                        nc.vector.tensor_copy(out=osb, in_=o_ps3[:, :])
                    nc.sync.dma_start(out=out_d[tb, :, osl], in_=osb)

                # one software pipeline across the whole attention phase:
                # den/pv (and each qb's normalization tail) lag the scores
                # matmul by 5 iterations so PE never waits on the exp (ACT).
                # During batch-1 attention, weave in phase-3 chains of batch 0
                # to keep PE fed through ACT-bound stretches.
                oo_work = [(tb, ob) for tb in range(TT) for ob in range(8)]
                woven = 0
                pend = []
                for b in range(B):
                    for h in range(HPC):
                        for qb in range(4):
                            for kt in range(4 * (qb + 1)):
                                pend.append(emit_s(b, h, qb, kt))
                                if len(pend) > 7:
                                    emit_dp(*pend.pop(0))
                            if b == 1:
                                # woven chains avoid the "den" tag: both den
                                # slots are held by live accumulators here
                                for _ in range(3):
                                    emit_oo(*oo_work[woven],
                                            tags=(("big", 4),), osb_dve=True)
                                    woven += 1
                for args in pend:
                    emit_dp(*args)
                while tails:
                    emit_tail(*tails.pop(0))

            # ---------------- Phase 3: output projection (remainder) -------
                for tb, ob in oo_work[woven:]:
                    emit_oo(tb, ob)
    nc.compile()
    return nc


def _rope_tables():
    freqs = np.einsum("i,j->ij", np.arange(MAX_POS),
                      1.0 / 10000 ** (np.arange(0, HD, 2) / HD)).astype("float32")
    emb = np.concatenate((freqs, freqs), axis=-1)  # [pos, HD]
    return np.sin(emb), np.cos(emb)


def _split8(x):
    """Split fp32 array into (hi, lo) fp8e4 planes, stacked on a new axis
    just before the last: [..., n] -> [..., 2, n] with j0=hi, j1=lo."""
    xc = np.clip(x, -240.0, 240.0)
    hi = xc.astype(NP_FP8)
    lo = (xc - hi.astype(np.float32)).astype(NP_FP8)
    return np.ascontiguousarray(np.stack((hi, lo), axis=-2))


def _prep_inputs(hidden_states, attention_mask, position_ids, Wq, Wk, Wv, Wo):
    hs = np.ascontiguousarray(np.asarray(hidden_states, dtype=np.float32))
    am = np.asarray(attention_mask, dtype=np.float32)
    pid = np.asarray(position_ids).astype(np.int64).reshape(-1)

    sin, cos = _rope_tables()
    sinT = np.ascontiguousarray(sin[pid].T)   # [HD, T]
    cosT = np.ascontiguousarray(cos[pid].T)
    sinT[0:HD // 2] *= -1.0                    # fold rotate-half sign
    sin_in = (sinT * (1.0 / WS)).astype(NP_BF16)   # undo weight prescale
    cos_in = (cosT * (1.0 / WS)).astype(NP_BF16)

    hsT = hs.reshape(T, HIDDEN).T                          # [HIDDEN, T]
    hsT = np.ascontiguousarray(
        hsT.reshape(KT, 128, T).transpose(1, 0, 2))        # [128, KT, T]
    # hidden hi/lo planes: [128, KT, 2, T]
    hh = hsT.astype(NP_FP8)
    hl = (hsT - hh.astype(np.float32)).astype(NP_FP8)
    hst_in = np.ascontiguousarray(np.stack((hh, hl), axis=2))

    # causal masks for diagonal blocks: allowed iff c >= 128*o + r
    r = np.arange(128)[:, None]
    c = np.arange(512)[None, :]
    masks = np.stack([np.where(c >= 128 * o + r, 0.0, NEG) for o in range(4)])
    masks = np.ascontiguousarray(masks.transpose(1, 0, 2)).astype(NP_BF16)  # [128,4,512]

    kb = np.where(am.reshape(-1) > 0, KB_SHIFT, NEG).astype(np.float32)
    kb_in = np.ascontiguousarray(kb.reshape(TT, 128).T)   # [128, TT]

    scale = 1.0 / math.sqrt(HD)
    Wq = np.asarray(Wq, dtype=np.float32) * (scale * WS)
    Wk = np.asarray(Wk, dtype=np.float32) * WS
    Wv = np.asarray(Wv, dtype=np.float32) * WS
    Wo = np.asarray(Wo, dtype=np.float32) * WS

    in_maps = []
    for m in range(NCORES):
        wq_m = np.ascontiguousarray(Wq[:, m * HPC * HD:(m + 1) * HPC * HD])
        wk_m = np.ascontiguousarray(Wk[:, m * HD:(m + 1) * HD])
        wv_m = np.ascontiguousarray(Wv[:, m * HD:(m + 1) * HD])
        wo_m = np.ascontiguousarray(Wo[m * HPC * HD:(m + 1) * HPC * HD, :])
        # per-kt (lo, hi) weight planes (cross instrs pair (lo,hi)x(hi,lo));
        # _split8 gives (hi, lo) so flip the plane axis
        wq8 = _split8(wq_m.reshape(KT, 128, HPC, HD).transpose(1, 2, 0, 3))
        wk8 = _split8(wk_m.reshape(KT, 128, HD).transpose(1, 0, 2))
        wv8 = _split8(wv_m.reshape(KT, 128, HD).transpose(1, 0, 2))
        wo8 = _split8(wo_m.reshape(HPC, 128, HIDDEN).transpose(1, 0, 2))
        in_maps.append({
            "hst": hst_in,
            "sint": sin_in,
            "cost": cos_in,
            # [128, HPC, KT, 2, HD]: partition-major, per-head blocked,
            # planes j0=lo j1=hi
            "wq": np.ascontiguousarray(wq8[:, :, :, ::-1, :]),
            "wk": np.ascontiguousarray(wk8[:, :, ::-1, :]),
            "wv": np.ascontiguousarray(wv8[:, :, ::-1, :]),
            # [128, HPC, 2, HIDDEN]: planes j0=hi j1=lo
            "wo": np.ascontiguousarray(wo8),
            "masks": masks,
            "kbias": kb_in,
        })
    return in_maps


def get_program():
    global _PROGRAM
    if _PROGRAM is None:
        _PROGRAM = _build_program()
    return _PROGRAM


def kernel(**inputs):
    nc = get_program()
    in_maps = _prep_inputs(
        inputs["hidden_states"], inputs["attention_mask"], inputs["position_ids"],
        inputs["Wq"], inputs["Wk"], inputs["Wv"], inputs["Wo"])
    res = run_bass_kernel_spmd(nc, in_maps, core_ids=list(range(NCORES)))
    acc = np.zeros((TT, 128, HIDDEN), dtype=np.float32)
    for r in res.results:
        acc += r["out"]
    return (acc * (1.0 / (SA * WS))).reshape(B, S, HIDDEN)
